# revision 8
# baseline (speedup 1.0000x reference)
"""DenseDilatedKnnGraph (B=4, C=64, N=8192, k=9, dilation=2) on 8 TRN2 NeuronCores.

Sharding: data-parallel over (batch, query-half): core i handles batch i//2,
query rows [ (i%2)*4096, (i%2+1)*4096 ), against all 8192 candidates.

Device (per 128-row tile, 32 tiles per core):
  1. 16 bf16 matmuls e = xb.T @ yb (K=64, N=512) -> PSUM f32.
  2. A 3-level pairwise-max reduction tree folds the 8192 candidate scores
     down to 1024 window-maxima (each window = 8 fixed columns), evicting
     PSUM->SBUF bf16 in the first level.  The tree levels are split across
     the DVE / GpSimd / ACT engines so all of PE, DVE, GpSimd and ACT run
     concurrently (~5us/tile each).
  3. DMA the [128, 1024] bf16 window maxima to HBM.

Host: per row, pick the top-K_WIN=48 windows by window max (argpartition),
expand to 384 candidate columns, rescore them exactly in f32
(dist = x_sq - 2*xb.yb + y_sq), sort by (dist, col) and keep even ranks
0,2,...,16 of the top-17.

Correctness guard (rigorous): every non-candidate column c has
bf16_window_max <= WK (the K-th best window max), so its true score satisfies
2e_c <= 2*(up(WK) + delta_e) with delta_e = 2^-8 + 2^-17 (bf16 input rounding
+ f32 accumulation, Cauchy-Schwarz on unit-norm rows).  If
x_sq - 2*(up(WK)+delta_e) + min(y_sq) could reach the 17th candidate dist the
row is recomputed exactly on the host (BLAS row x full yb).  On the graded
input zero rows get flagged (validated in simulation with ~2 near-tie
mismatches from f32 rescore rounding, rel err ~5e-4 << 2e-2).
"""

import os
import sys

import numpy as np


def _ensure_concourse():
    try:
        import concourse.bass  # noqa: F401
    except ImportError:
        for p in (
            "/root/.axon_site",
            "/root/.axon_site/_ro/trn_rl_repo",
            "/root/.axon_site/_ro/pypackages",
            "/opt/trn_rl_repo",
            "/opt/pypackages",
        ):
            if os.path.isdir(p) and p not in sys.path:
                sys.path.append(p)


_ensure_concourse()

import jax.numpy as jnp  # noqa: E402
import ml_dtypes  # noqa: E402

import concourse.bacc as bacc  # noqa: E402
import concourse.mybir as mybir  # noqa: E402
from concourse.bass_utils import run_bass_kernel_spmd  # noqa: E402
from concourse.tile import TileContext  # noqa: E402

BF = ml_dtypes.bfloat16

B, C, N = 4, 64, 8192
K_NEIGHBORS, DILATION = 9, 2
TOPK = 17                            # ranks 0..16; even ones are kept
EPS = 1e-12

NCORES = 8
ROWS = N // 2                        # query rows per core
TILE_P = 128
NT = ROWS // TILE_P                  # 32 row-tiles per core
MM_N = 512
NMM = N // MM_N                      # 16 matmuls per row-tile
NWIN = 1024                          # window maxima per row after the tree
WSZ = N // NWIN                      # 8 columns per window
K_WIN = 48                           # windows rescored per row on the host

# engine assignment knobs (tuned against TimelineSim):
#   L1: 8 units; unit u pairs PSUM banks (2u, 2u+1) -> W1[:, 512u:512u+512].
#   HW rule NCC_IBVF027: a TT op may read at most ONE input from PSUM, so
#   ACT always evicts at least one bank of the pair to bf16 SBUF first:
#       "h_dve"/"h_gp":     ACT copies bank 2u;  TT(psum bank 2u+1, copy)
#       "act_dve"/"act_gp": ACT copies both banks; TT on the bf16 pair
#   L2: two 1024-wide halves: W2[h] = max(W1[h], W1[h+2048])
#   L3: two 512-wide halves:  W3[h] = max(W2[h], W2[h+1024])
L1_ASSIGN = ("h_dve", "h_gp", "h_dve", "h_gp", "h_dve", "h_gp", "h_dve", "h_gp")
L2_ENG = ("dve", "gp")
L3_ENG = ("dve", "dve")

_BUILT = None


def _build_bass():
    f32, bf16 = mybir.dt.float32, mybir.dt.bfloat16
    nc = bacc.Bacc("TRN2", target_bir_lowering=False, debug=False)

    la_d = nc.dram_tensor("la", [C, ROWS], bf16, kind="ExternalInput")
    ra_d = nc.dram_tensor("ra", [C, N], bf16, kind="ExternalInput")
    w3_d = nc.dram_tensor("w3", [ROWS, NWIN], bf16, kind="ExternalOutput")

    # bf16 staging slots: act_* units evict both banks (1024), h_* one (512)
    ev_off = []
    tot = 0
    for a in L1_ASSIGN:
        ev_off.append(tot)
        tot += 1024 if a.startswith("act") else 512
    n_ev = tot

    with TileContext(nc) as tc:
        with (
            tc.tile_pool(name="weights", bufs=1) as wpool,
            tc.tile_pool(name="work", bufs=2) as wk,
            tc.tile_pool(name="psum", bufs=1, space="PSUM") as psum,
        ):
            LA = wpool.tile([C, ROWS], bf16)
            RA = wpool.tile([C, N], bf16)
            for j in range(NMM):
                sl = slice(j * MM_N, (j + 1) * MM_N)
                nc.sync.dma_start(RA[:, sl], ra_d[:, sl])
                if (j + 1) * MM_N <= ROWS:
                    nc.sync.dma_start(LA[:, sl], la_d[:, sl])

            for mt in range(NT):
                lhsT = LA[:, mt * TILE_P : (mt + 1) * TILE_P]
                W1 = wk.tile([TILE_P, 4096], bf16, tag="W1")
                W2 = wk.tile([TILE_P, 2048], bf16, tag="W2")
                W3 = wk.tile([TILE_P, NWIN], bf16, tag="W3")
                BEV = wk.tile([TILE_P, n_ev], bf16, tag="BEV", name="BEV")

                ps = []
                for j in range(NMM):
                    p = psum.tile([TILE_P, MM_N], f32, tag=f"b{j % 8}")
                    nc.tensor.matmul(
                        p[:], lhsT, RA[:, j * MM_N : (j + 1) * MM_N],
                        start=True, stop=True,
                    )
                    ps.append(p)

                for u, asg in enumerate(L1_ASSIGN):
                    o = W1[:, 512 * u : 512 * (u + 1)]
                    a, b = ps[2 * u], ps[2 * u + 1]
                    eng = nc.vector if asg.endswith("dve") else nc.gpsimd
                    if asg.startswith("h_"):
                        bev = BEV[:, ev_off[u] : ev_off[u] + 512]
                        _t = nc.scalar.activation(
                            bev, a[:], mybir.ActivationFunctionType.Copy
                        )
                        _t = eng.tensor_max(o, b[:], bev)
                    else:
                        bev = BEV[:, ev_off[u] : ev_off[u] + 1024]
                        _t = nc.scalar.activation(
                            bev[:, 0:512], a[:],
                            mybir.ActivationFunctionType.Copy,
                        )
                        _t = nc.scalar.activation(
                            bev[:, 512:1024], b[:],
                            mybir.ActivationFunctionType.Copy,
                        )
                        _t = eng.tensor_max(o, bev[:, 0:512], bev[:, 512:1024])

                for h, eng_name in enumerate(L2_ENG):
                    eng = nc.vector if eng_name == "dve" else nc.gpsimd
                    sl = slice(1024 * h, 1024 * (h + 1))
                    sl2 = slice(2048 + 1024 * h, 2048 + 1024 * (h + 1))
                    _t = eng.tensor_max(W2[:, sl], W1[:, sl], W1[:, sl2])

                for h, eng_name in enumerate(L3_ENG):
                    eng = nc.vector if eng_name == "dve" else nc.gpsimd
                    sl = slice(512 * h, 512 * (h + 1))
                    sl2 = slice(1024 + 512 * h, 1024 + 512 * (h + 1))
                    _t = eng.tensor_max(W3[:, sl], W2[:, sl], W2[:, sl2])

                rows = slice(mt * TILE_P, (mt + 1) * TILE_P)
                nc.sync.dma_start(w3_d[rows, :], W3[:])

    nc.compile()
    return nc


def _norm_feats(v):
    """The reference's exact normalization expressions."""
    v = jnp.asarray(v)
    nrm = jnp.sqrt(jnp.sum(v * v, axis=1, keepdims=True))
    vn = v / jnp.maximum(nrm, EPS)
    vb = jnp.squeeze(vn, -1).transpose(0, 2, 1)      # [B, N, C]
    sq = jnp.sum(vb * vb, axis=-1)                   # [B, N]
    return np.asarray(vb), np.asarray(sq)


def _window_lut():
    """col -> window mapping of the 3-level pair tree; returns [NWIN, WSZ]."""
    c = np.arange(N)
    w1 = 512 * (c // 1024) + (c % 512)
    w3 = (w1 % 2048) % 1024
    order = np.argsort(w3, kind="stable")
    return order.reshape(NWIN, WSZ)


_LUT = _window_lut()
_DELTA_E = 2.0 ** -8 + 2.0 ** -17


def kernel(x: np.ndarray, y: np.ndarray) -> np.ndarray:
    global _BUILT
    if _BUILT is None:
        _BUILT = _build_bass()
    nc = _BUILT

    x = np.asarray(x)
    y = np.asarray(y)
    xb, x_sq = _norm_feats(x)
    yb, y_sq = _norm_feats(y)
    la_all = np.ascontiguousarray(xb.transpose(0, 2, 1)).astype(BF)   # [B, C, N]
    ra_all = np.ascontiguousarray(yb.transpose(0, 2, 1)).astype(BF)

    in_maps = []
    for core in range(NCORES):
        b, half = core >> 1, core & 1
        cols = slice(half * ROWS, (half + 1) * ROWS)
        in_maps.append(
            {
                "la": np.ascontiguousarray(la_all[b][:, cols]),
                "ra": np.ascontiguousarray(ra_all[b]),
            }
        )

    try:
        res = run_bass_kernel_spmd(nc, in_maps, list(range(NCORES)))
    except Exception:
        import time

        time.sleep(2.0)
        res = run_bass_kernel_spmd(nc, in_maps, list(range(NCORES)))

    nn_idx = np.empty((B, N, TOPK), np.int64)
    for core in range(NCORES):
        b, half = core >> 1, core & 1
        w3 = np.asarray(res.results[core]["w3"]).astype(np.float32)  # [ROWS, 1024]

        part = np.argpartition(-w3, K_WIN, axis=1)[:, :K_WIN]
        wk = -np.partition(-w3, K_WIN, axis=1)[:, K_WIN - 1]         # K-th best
        cand = _LUT[part].reshape(ROWS, K_WIN * WSZ)                 # [ROWS, 384]

        rows_blk = slice(half * ROWS, (half + 1) * ROWS)
        xb_c = xb[b][rows_blk]                                       # [ROWS, C]
        xsq_c = x_sq[b][rows_blk]

        e_ex = np.empty((ROWS, K_WIN * WSZ), np.float32)
        for i0 in range(0, ROWS, 1024):
            sl = slice(i0, i0 + 1024)
            g = yb[b][cand[sl]]                                      # [1024, 384, C]
            e_ex[sl] = np.einsum("rkc,rc->rk", g, xb_c[sl], optimize=True)
        dist = (xsq_c[:, None] - 2.0 * e_ex + y_sq[b][cand]).astype(np.float32)
        order = np.lexsort((cand, dist), axis=-1)[:, :TOPK]
        top = np.take_along_axis(cand, order, axis=1)
        d17 = np.take_along_axis(dist, order[:, TOPK - 1 : TOPK], axis=1)[:, 0]

        # guard: can any excluded column beat the 17th candidate?
        up = wk + np.abs(wk) * 2.0 ** -8 + 1e-30
        dist_excl_min = xsq_c - 2.0 * (up + _DELTA_E) + y_sq[b].min()
        bad = np.flatnonzero(
            dist_excl_min <= d17 + 4e-7 * np.maximum(1.0, np.abs(d17))
        )
        if bad.size:
            e_full = xb_c[bad] @ yb[b].T
            dist_full = (
                xsq_c[bad, None] - 2.0 * e_full + y_sq[b][None, :]
            ).astype(np.float32)
            ordf = np.lexsort(
                (np.broadcast_to(np.arange(N), dist_full.shape), dist_full),
                axis=-1,
            )[:, :TOPK]
            top[bad] = ordf

        nn_idx[b, rows_blk] = top

    nn_keep = nn_idx[:, :, 0:TOPK:DILATION].astype(np.int32)         # [B, N, 9]
    center = np.broadcast_to(
        np.arange(N, dtype=np.int32)[None, :, None], (B, N, K_NEIGHBORS)
    )
    return np.stack((nn_keep, center), axis=0)                       # [2, B, N, 9]


# revision 9
# speedup vs baseline: 1.1079x; 1.1079x over previous
"""DenseDilatedKnnGraph (B=4, C=64, N=8192, k=9, dilation=2) on 8 TRN2 NeuronCores.

Sharding: data-parallel over (batch, query-half): core i handles batch i//2,
query rows [ (i%2)*4096, (i%2+1)*4096 ), against all 8192 candidates.

Device (per 128-row tile, 32 tiles per core):
  1. 16 bf16 matmuls e = xb.T @ yb (K=64, N=512) -> PSUM f32 (8-bank rotation).
  2. A pairwise-max reduction tree folds the 8192 scores down to
     8192 >> OUT_LEVEL bf16 window maxima.  Level 1 evicts PSUM: the HW
     allows at most one PSUM operand per vector op (NCC_IBVF027), so a
     copy engine (ACT/DVE/GP, per-unit knob) evicts one bank of each pair
     to bf16 SBUF and a TT-max engine (DVE/GP knob) folds the other PSUM
     bank against it.  Levels 2/3 are bf16 TT-max on DVE/GP.  The knobs
     spread the work so PE/DVE/GP/ACT all run concurrently.
  3. DMA the [128, 8192>>OUT_LEVEL] bf16 window maxima to HBM.

Host: per row, pick the top-K_WIN windows by window max (argpartition),
expand to K_WIN << OUT_LEVEL candidate columns, rescore them exactly in f32
(dist = x_sq - 2*xb.yb + y_sq), sort by (dist, col) and keep even ranks
0,2,...,16 of the top-17.

Correctness guard (rigorous): every non-candidate column c has
bf16_window_max <= WK (the K-th best window max), so its true score satisfies
e_c <= up(WK) + delta_e with delta_e = 2^-8 + 2^-17 (bf16 input rounding +
f32 accumulation, Cauchy-Schwarz on unit-norm rows).  If
x_sq - 2*(up(WK)+delta_e) + min(y_sq) could reach the 17th candidate dist the
row is recomputed exactly on the host (BLAS row x full yb).  On the graded
input zero rows get flagged (validated in simulation; ~2 near-tie mismatches
from f32 rescore rounding, rel err ~5e-4 << 2e-2).
"""

import os
import sys

import numpy as np


def _ensure_concourse():
    try:
        import concourse.bass  # noqa: F401
    except ImportError:
        for p in (
            "/root/.axon_site",
            "/root/.axon_site/_ro/trn_rl_repo",
            "/root/.axon_site/_ro/pypackages",
            "/opt/trn_rl_repo",
            "/opt/pypackages",
        ):
            if os.path.isdir(p) and p not in sys.path:
                sys.path.append(p)


_ensure_concourse()

import jax.numpy as jnp  # noqa: E402
import ml_dtypes  # noqa: E402

import concourse.bacc as bacc  # noqa: E402
import concourse.mybir as mybir  # noqa: E402
from concourse.bass_utils import run_bass_kernel_spmd  # noqa: E402
from concourse.tile import TileContext  # noqa: E402

BF = ml_dtypes.bfloat16

B, C, N = 4, 64, 8192
K_NEIGHBORS, DILATION = 9, 2
TOPK = 17                            # ranks 0..16; even ones are kept
EPS = 1e-12

NCORES = 8
ROWS = N // 2                        # query rows per core
TILE_P = 128
NT = ROWS // TILE_P                  # 32 row-tiles per core
MM_N = 512
NMM = N // MM_N                      # 16 matmuls per row-tile
K_WIN = 48                           # windows rescored per row on the host

# ---- tuning knobs (validated against TimelineSim) ----
OUT_LEVEL = 2                        # tree depth on device (1, 2 or 3)
# L1 unit u pairs PSUM banks (2u, 2u+1): (evict engine, TT engine)
UNITS = (
    ("act", "dve"), ("act", "gp"), ("act", "dve"), ("act", "gp"),
    ("act", "dve"), ("act", "gp"), ("act", "dve"), ("act", "gp"),
)
L2_ENG = ("dve", "dve")              # two 1024-wide halves (if OUT_LEVEL >= 2)
L3_ENG = ("dve", "dve")              # two 512-wide halves (if OUT_LEVEL == 3)

NWIN = N >> OUT_LEVEL                # windows per row shipped to the host
WSZ = N // NWIN                      # columns per window

_BUILT = None


def _build_bass():
    f32, bf16 = mybir.dt.float32, mybir.dt.bfloat16
    nc = bacc.Bacc("TRN2", target_bir_lowering=False, debug=False)

    la_d = nc.dram_tensor("la", [C, ROWS], bf16, kind="ExternalInput")
    ra_d = nc.dram_tensor("ra", [C, N], bf16, kind="ExternalInput")
    w_d = nc.dram_tensor("w", [ROWS, NWIN], bf16, kind="ExternalOutput")

    def engine(name):
        return nc.vector if name == "dve" else nc.gpsimd

    with TileContext(nc) as tc:
        with (
            tc.tile_pool(name="weights", bufs=1) as wpool,
            tc.tile_pool(name="work", bufs=2) as wk,
            tc.tile_pool(name="psum", bufs=1, space="PSUM") as psum,
        ):
            LA = wpool.tile([C, ROWS], bf16)
            RA = wpool.tile([C, N], bf16)
            for j in range(NMM):
                sl = slice(j * MM_N, (j + 1) * MM_N)
                nc.sync.dma_start(RA[:, sl], ra_d[:, sl])
                if (j + 1) * MM_N <= ROWS:
                    nc.sync.dma_start(LA[:, sl], la_d[:, sl])

            for mt in range(NT):
                lhsT = LA[:, mt * TILE_P : (mt + 1) * TILE_P]
                W1 = wk.tile([TILE_P, 4096], bf16, tag="W1", name="W1")
                BEV = wk.tile([TILE_P, 4096], bf16, tag="BEV", name="BEV")

                ps = []
                for j in range(NMM):
                    p = psum.tile(
                        [TILE_P, MM_N], f32, tag=f"b{j % 8}", name=f"psb{j % 8}"
                    )
                    nc.tensor.matmul(
                        p[:], lhsT, RA[:, j * MM_N : (j + 1) * MM_N],
                        start=True, stop=True,
                    )
                    ps.append(p)

                for u, (ev, tt) in enumerate(UNITS):
                    o = W1[:, 512 * u : 512 * (u + 1)]
                    a, b = ps[2 * u], ps[2 * u + 1]
                    bev = BEV[:, 512 * u : 512 * (u + 1)]
                    if ev == "act":
                        _t = nc.scalar.activation(
                            bev, a[:], mybir.ActivationFunctionType.Copy
                        )
                    else:
                        _t = engine(ev).tensor_copy(bev, a[:])
                    _t = engine(tt).tensor_max(o, b[:], bev)

                OUT = W1
                if OUT_LEVEL >= 2:
                    W2 = wk.tile([TILE_P, 2048], bf16, tag="W2", name="W2")
                    for h, en in enumerate(L2_ENG):
                        sl = slice(1024 * h, 1024 * (h + 1))
                        sl2 = slice(2048 + 1024 * h, 2048 + 1024 * (h + 1))
                        _t = engine(en).tensor_max(W2[:, sl], W1[:, sl], W1[:, sl2])
                    OUT = W2
                if OUT_LEVEL >= 3:
                    W3 = wk.tile([TILE_P, 1024], bf16, tag="W3", name="W3")
                    for h, en in enumerate(L3_ENG):
                        sl = slice(512 * h, 512 * (h + 1))
                        sl2 = slice(1024 + 512 * h, 1024 + 512 * (h + 1))
                        _t = engine(en).tensor_max(W3[:, sl], W2[:, sl], W2[:, sl2])
                    OUT = W3

                rows = slice(mt * TILE_P, (mt + 1) * TILE_P)
                nc.sync.dma_start(w_d[rows, :], OUT[:])

    nc.compile()
    return nc


def _norm_feats(v):
    """The reference's exact normalization expressions."""
    v = jnp.asarray(v)
    nrm = jnp.sqrt(jnp.sum(v * v, axis=1, keepdims=True))
    vn = v / jnp.maximum(nrm, EPS)
    vb = jnp.squeeze(vn, -1).transpose(0, 2, 1)      # [B, N, C]
    sq = jnp.sum(vb * vb, axis=-1)                   # [B, N]
    return np.asarray(vb), np.asarray(sq)


def _window_lut():
    """col -> window mapping of the pair tree; returns [NWIN, WSZ]."""
    c = np.arange(N)
    w = 512 * (c // 1024) + (c % 512)            # level 1 (width 4096)
    if OUT_LEVEL >= 2:
        w = w % 2048
    if OUT_LEVEL >= 3:
        w = w % 1024
    order = np.argsort(w, kind="stable")
    return order.reshape(NWIN, WSZ)


_LUT = _window_lut()
_DELTA_E = 2.0 ** -8 + 2.0 ** -17


def kernel(x: np.ndarray, y: np.ndarray) -> np.ndarray:
    global _BUILT
    if _BUILT is None:
        _BUILT = _build_bass()
    nc = _BUILT

    x = np.asarray(x)
    y = np.asarray(y)
    xb, x_sq = _norm_feats(x)
    yb, y_sq = _norm_feats(y)
    la_all = np.ascontiguousarray(xb.transpose(0, 2, 1)).astype(BF)   # [B, C, N]
    ra_all = np.ascontiguousarray(yb.transpose(0, 2, 1)).astype(BF)

    in_maps = []
    for core in range(NCORES):
        b, half = core >> 1, core & 1
        cols = slice(half * ROWS, (half + 1) * ROWS)
        in_maps.append(
            {
                "la": np.ascontiguousarray(la_all[b][:, cols]),
                "ra": np.ascontiguousarray(ra_all[b]),
            }
        )

    try:
        res = run_bass_kernel_spmd(nc, in_maps, list(range(NCORES)))
    except Exception:
        import time

        time.sleep(2.0)
        res = run_bass_kernel_spmd(nc, in_maps, list(range(NCORES)))

    nn_idx = np.empty((B, N, TOPK), np.int64)
    for core in range(NCORES):
        b, half = core >> 1, core & 1
        w = np.asarray(res.results[core]["w"]).astype(np.float32)    # [ROWS, NWIN]

        part = np.argpartition(-w, K_WIN, axis=1)[:, :K_WIN]
        wkth = -np.partition(-w, K_WIN, axis=1)[:, K_WIN - 1]        # K-th best
        cand = _LUT[part].reshape(ROWS, K_WIN * WSZ)

        rows_blk = slice(half * ROWS, (half + 1) * ROWS)
        xb_c = xb[b][rows_blk]                                       # [ROWS, C]
        xsq_c = x_sq[b][rows_blk]

        e_ex = np.empty((ROWS, K_WIN * WSZ), np.float32)
        for i0 in range(0, ROWS, 1024):
            sl = slice(i0, i0 + 1024)
            g = yb[b][cand[sl]]                                      # [1024, nc, C]
            e_ex[sl] = np.einsum("rkc,rc->rk", g, xb_c[sl], optimize=True)
        dist = (xsq_c[:, None] - 2.0 * e_ex + y_sq[b][cand]).astype(np.float32)
        order = np.lexsort((cand, dist), axis=-1)[:, :TOPK]
        top = np.take_along_axis(cand, order, axis=1)
        d17 = np.take_along_axis(dist, order[:, TOPK - 1 : TOPK], axis=1)[:, 0]

        # guard: can any excluded column beat the 17th candidate?
        up = wkth + np.abs(wkth) * 2.0 ** -8 + 1e-30
        dist_excl_min = xsq_c - 2.0 * (up + _DELTA_E) + y_sq[b].min()
        bad = np.flatnonzero(
            dist_excl_min <= d17 + 4e-7 * np.maximum(1.0, np.abs(d17))
        )
        if bad.size:
            e_full = xb_c[bad] @ yb[b].T
            dist_full = (
                xsq_c[bad, None] - 2.0 * e_full + y_sq[b][None, :]
            ).astype(np.float32)
            ordf = np.lexsort(
                (np.broadcast_to(np.arange(N), dist_full.shape), dist_full),
                axis=-1,
            )[:, :TOPK]
            top[bad] = ordf

        nn_idx[b, rows_blk] = top

    nn_keep = nn_idx[:, :, 0:TOPK:DILATION].astype(np.int32)         # [B, N, 9]
    center = np.broadcast_to(
        np.arange(N, dtype=np.int32)[None, :, None], (B, N, K_NEIGHBORS)
    )
    return np.stack((nn_keep, center), axis=0)                       # [2, B, N, 9]
